# revision 2
# baseline (speedup 1.0000x reference)
"""Trainium2 Bass kernel for nn_Differ (pairwise mu/Sigma differences).

Full-input contract: kernel(mu, Sigma) -> (mu_d, sig_d), each [N*N] f32.

  off-diag (j != k): mu_d[j,k] = mu[j] - mu[k]
                     sig_d[j,k] = S[j,j] + S[k,k] - 2*S[j,k]
  diag     (j == k): mu_d[j,j] = -mu[j]
                     sig_d[j,j] = S[j,j]

Sharding: the j (row) axis of the N x N pairwise grid is split into 8
contiguous blocks of 512 rows, one per NeuronCore.  Each core reads its
512 Sigma rows plus diag(Sigma) and mu, and writes its [512, 4096] block
of both outputs.  The N diagonal elements are overwritten on the host
during unsharding (the device formula gives exactly 0.0 there), which
keeps the SPMD program identical across cores.

Measured design notes:
  - Traffic floor: 8.4 MiB Sigma in + 16.8 MiB out per core; the 16
    SDMA engines pool ~425 GB/s and per-NC HBM is ~358 GB/s, so the
    kernel is a pure streaming problem -- keep HBM saturated from ~1us.
  - Both row vectors (d and mu) are replicated across partitions by the
    idle TensorEngine as ones[128]^T @ x (bitwise-exact for fp32 on HW),
    so only ~37 KiB of vector data is read from HBM.  Both broadcasts
    stay RESIDENT in PSUM; since 2 x [128, 4096] f32 would be 2x PSUM
    capacity, the column axis is processed in two 2048-wide phases with
    mu_ps and d_ps each holding 4 banks.  The phase-B matmuls rebuild
    the broadcasts as soon as the phase-A ACTs have read them (WAR
    handled by Tile), overlapping with phase-A's sig pass.
  - HWDGE rings are FIFO per issuing engine; loads + sig stores ride
    the sync (SP) ring (loads finish early, then the ring streams sig
    tiles), mu stores ride the scalar (ACT) ring from ~5us.  SWDGE and
    SBUF->SBUF replication DMAs are <100 GB/s here -- avoided.
  - Work pools are 5 tiles deep per output stream so no ACT ever blocks
    on a store-completion round trip (the 85us baseline's failure mode).
"""

import numpy as np

N = 4096
NCORES = 8
RPC = N // NCORES  # 512 rows per core
P = 128            # SBUF partitions
TILES = RPC // P   # 4 row-tiles per core
BANK = 512         # fp32 elements per PSUM bank (matmul N limit)
HALF = N // 2      # column-phase width (2 PSUM residents of 4 banks)

_PROGRAM = None


def _build_program():
    import concourse.bacc as bacc
    import concourse.mybir as mybir
    import concourse.tile as tile
    from concourse.bass import get_trn_type

    f32 = mybir.dt.float32
    ident = mybir.ActivationFunctionType.Identity

    # Bacc (not raw Bass): its generate_event_semaphores pass splits
    # multi-semaphore waits, which TRN2 engines cannot encode (walrus
    # rejects >1 sync wait per instruction).
    nc = bacc.Bacc(
        get_trn_type() or "TRN2",
        target_bir_lowering=False,
        debug=False,
        num_devices=NCORES,
    )
    sigma = nc.declare_dram_parameter("sigma_rows", [RPC, N], f32, isOutput=False)
    # xsvec = [diag(Sigma) || ones(128) || mu]
    xsvec = nc.declare_dram_parameter("xsvec", [1, 2 * N + P], f32, isOutput=False)
    # cols[r, t] = d[j0 + t*128 + r], cols[r, TILES+t] = mu[j0 + t*128 + r]
    cols = nc.declare_dram_parameter("cols", [P, 2 * TILES], f32, isOutput=False)
    mu_out = nc.declare_dram_parameter("mu_out", [RPC, N], f32, isOutput=True)
    sig_out = nc.declare_dram_parameter("sig_out", [RPC, N], f32, isOutput=True)

    with tile.TileContext(nc) as tc:
        with (
            tc.tile_pool(name="const", bufs=1) as cpool,
            tc.tile_pool(name="psum", bufs=1, space="PSUM") as ppool,
            tc.tile_pool(name="work", bufs=1) as work,
        ):
            # xs = [d || ones || mu], one DMA so no slot-reuse WAR can
            # ever block the sync ring's prefetch stream.
            xs = cpool.tile([1, 2 * N + P], f32, tag="xs")
            cols_sb = cpool.tile([P, 2 * TILES], f32, tag="cols")

            nc.sync.dma_start(out=xs[:], in_=xsvec[0:1, :])
            nc.sync.dma_start(out=cols_sb[:], in_=cols[:, :])
            # Prefetch all sigma tiles up front (bufs=4 -> no slot waits).
            s_tiles = []
            for t in range(TILES):
                s = work.tile([P, N], f32, tag="s", bufs=TILES)
                nc.sync.dma_start(out=s[:], in_=sigma[t * P:(t + 1) * P, :])
                s_tiles.append(s)

            ones = xs[0:1, N:N + P]

            # Both broadcasts live in PSUM, 4 banks each, rebuilt per
            # column phase: ones[1,128]^T @ x[1,512] per bank (bitwise
            # exact for fp32, verified on HW).
            mu_ps = ppool.tile([P, HALF], f32, tag="mups")
            d_ps = ppool.tile([P, HALF], f32, tag="dps")

            for h in range(2):
                c0 = h * HALF
                # mu broadcast first: the mu pass has no sigma dep, so
                # its stores feed the scalar ring from ~5us.
                for c in range(HALF // BANK):
                    nc.tensor.matmul(
                        mu_ps[:, c * BANK:(c + 1) * BANK], ones,
                        xs[0:1, N + P + c0 + c * BANK:N + P + c0 + (c + 1) * BANK],
                        start=True, stop=True,
                    )
                for c in range(HALF // BANK):
                    nc.tensor.matmul(
                        d_ps[:, c * BANK:(c + 1) * BANK], ones,
                        xs[0:1, c0 + c * BANK:c0 + (c + 1) * BANK],
                        start=True, stop=True,
                    )

                # mu pass: m = -mu_k + mu_j, straight from PSUM.
                for t in range(TILES):
                    m = work.tile([P, HALF], f32, tag="m", bufs=5)
                    nc.scalar.activation(
                        m[:], mu_ps[:], ident,
                        bias=cols_sb[:, TILES + t:TILES + t + 1], scale=-1.0,
                    )
                    nc.scalar.dma_start(
                        out=mu_out[t * P:(t + 1) * P, c0:c0 + HALF], in_=m[:]
                    )

                # sig pass: T = d_k + d_j (from PSUM), then one fused DVE
                # op sig = (S * -2) + T; -2*S is exact and T + (-2S)
                # rounds identically to T - 2S, so this stays bitwise
                # equal to the reference.
                for t in range(TILES):
                    tt = work.tile([P, HALF], f32, tag="tt", bufs=5)
                    nc.scalar.activation(
                        tt[:], d_ps[:], ident,
                        bias=cols_sb[:, t:t + 1], scale=1.0,
                    )
                    nc.vector.scalar_tensor_tensor(
                        tt[:], s_tiles[t][:, c0:c0 + HALF], -2.0, tt[:],
                        op0=mybir.AluOpType.mult, op1=mybir.AluOpType.add,
                    )
                    # sig stores ride the sync ring, which is done
                    # loading by the time these are ready.
                    nc.sync.dma_start(
                        out=sig_out[t * P:(t + 1) * P, c0:c0 + HALF], in_=tt[:]
                    )

    return nc


def _get_program():
    global _PROGRAM
    if _PROGRAM is None:
        nc = _build_program()
        # Bacc defers register allocation / wait splitting to finalize();
        # the axon PJRT path serializes the module as-is, so run it here.
        nc.finalize()
        _PROGRAM = nc
    return _PROGRAM


def _make_in_maps(mu, Sigma, d):
    xsvec = np.concatenate([d, np.ones(P, np.float32), mu]).reshape(1, 2 * N + P)
    in_maps = []
    for c in range(NCORES):
        j0 = c * RPC
        cols = np.concatenate(
            [
                d[j0:j0 + RPC].reshape(TILES, P).T,
                mu[j0:j0 + RPC].reshape(TILES, P).T,
            ],
            axis=1,
        )
        in_maps.append({
            "sigma_rows": np.ascontiguousarray(Sigma[j0:j0 + RPC]),
            "xsvec": xsvec,
            "cols": np.ascontiguousarray(cols),
        })
    return in_maps


def _assemble(per_core_results, mu, d):
    mu_full = np.concatenate(
        [per_core_results[c]["mu_out"] for c in range(NCORES)], axis=0
    )
    sig_full = np.concatenate(
        [per_core_results[c]["sig_out"] for c in range(NCORES)], axis=0
    )
    idx = np.arange(N)
    mu_full[idx, idx] = -mu
    sig_full[idx, idx] = d
    return mu_full.reshape(-1), sig_full.reshape(-1)


def kernel(mu, Sigma, _trace=False):
    from concourse.bass_utils import run_bass_kernel_spmd

    mu = np.ascontiguousarray(np.asarray(mu, dtype=np.float32).reshape(N))
    Sigma = np.ascontiguousarray(np.asarray(Sigma, dtype=np.float32).reshape(N, N))
    d = np.ascontiguousarray(np.diagonal(Sigma)).astype(np.float32)

    nc = _get_program()
    in_maps = _make_in_maps(mu, Sigma, d)
    res = run_bass_kernel_spmd(nc, in_maps, list(range(NCORES)), trace=_trace)
    out = _assemble(res.results, mu, d)
    if _trace:
        return out, res
    return out


# revision 6
# speedup vs baseline: 1.1095x; 1.1095x over previous
"""Trainium2 Bass kernel for nn_Differ (pairwise mu/Sigma differences).

Full-input contract: kernel(mu, Sigma) -> (mu_d, sig_d), each [N*N] f32.

  off-diag (j != k): mu_d[j,k] = mu[j] - mu[k]
                     sig_d[j,k] = S[j,j] + S[k,k] - 2*S[j,k]
  diag     (j == k): mu_d[j,j] = -mu[j]
                     sig_d[j,j] = S[j,j]

Sharding: the j (row) axis of the N x N pairwise grid is split into 8
contiguous blocks of 512 rows, one per NeuronCore.  Each core reads its
512 Sigma rows plus diag(Sigma) and mu, and writes its [512, 4096] block
of both outputs.  The N diagonal elements are overwritten on the host
during unsharding (the device formula gives exactly 0.0 there), which
keeps the SPMD program identical across cores.

Measured design notes:
  - Traffic floor: 8.4 MiB Sigma in + 16.8 MiB out per core; the 16
    SDMA engines pool ~425 GB/s and the kernel sustains ~420 GB/s
    mid-stream, so everything rides on keeping the pool fed end-to-end.
  - Row vectors (d, -mu) are replicated across partitions by the idle
    TensorEngine as w[1,128]^T @ x (bitwise-exact for fp32 on HW; the
    -ones weight makes mu_ps hold -mu_k so the mu pass is a single add).
    Both broadcasts stay RESIDENT in PSUM; 2 x [128, 4096] f32 would be
    2x PSUM capacity, so the column axis runs in two 2048-wide phases
    (WAR on the PSUM tiles is tracked by Tile).
  - Compute is split so no engine gates the store stream: Vector does
    the mu pass (tensor_scalar_add with a [P,1] per-row bias; GpSimd
    cannot read PSUM) plus the fused sig = (S * -2) + T, Scalar does
    the sig bias-add from PSUM.  Compute finishes by ~52us while the
    DMA pool needs ~60us -- the kernel is purely DMA-bound.
  - mu halves are staged into full-width [128, 4096] tiles and stored
    as 4 DMAs (16 KiB lines); sig goes out as 8 half-width stores.
    Loads + sig stores ride the sync HWDGE ring, xs/cols + mu stores
    ride the scalar ring, so the first bytes move at ~7us on both.
"""

import numpy as np

N = 4096
NCORES = 8
RPC = N // NCORES  # 512 rows per core
P = 128            # SBUF partitions
TILES = RPC // P   # 4 row-tiles per core
BANK = 512         # fp32 elements per PSUM bank (matmul N limit)
HALF = N // 2      # column-phase width (2 PSUM residents of 4 banks)

_PROGRAM = None


def _build_program():
    import concourse.bacc as bacc
    import concourse.mybir as mybir
    import concourse.tile as tile
    from concourse.bass import get_trn_type

    f32 = mybir.dt.float32
    ident = mybir.ActivationFunctionType.Identity

    # Bacc (not raw Bass): its generate_event_semaphores pass splits
    # multi-semaphore waits, which TRN2 engines cannot encode (walrus
    # rejects >1 sync wait per instruction).
    nc = bacc.Bacc(
        get_trn_type() or "TRN2",
        target_bir_lowering=False,
        debug=False,
        num_devices=NCORES,
    )
    sigma = nc.declare_dram_parameter("sigma_rows", [RPC, N], f32, isOutput=False)
    # xsvec = [diag(Sigma) || ones(128) || -ones(128) || mu]
    xsvec = nc.declare_dram_parameter("xsvec", [1, 2 * N + 2 * P], f32, isOutput=False)
    # cols[r, t] = d[j0 + t*128 + r], cols[r, TILES+t] = mu[j0 + t*128 + r]
    cols = nc.declare_dram_parameter("cols", [P, 2 * TILES], f32, isOutput=False)
    mu_out = nc.declare_dram_parameter("mu_out", [RPC, N], f32, isOutput=True)
    sig_out = nc.declare_dram_parameter("sig_out", [RPC, N], f32, isOutput=True)

    with tile.TileContext(nc) as tc:
        with (
            tc.tile_pool(name="const", bufs=1) as cpool,
            tc.tile_pool(name="psum", bufs=1, space="PSUM") as ppool,
            tc.tile_pool(name="work", bufs=1) as work,
        ):
            # xs = [d || ones || -ones || mu] in one DMA so no slot-reuse
            # WAR can ever block a ring's prefetch stream.
            xs = cpool.tile([1, 2 * N + 2 * P], f32, tag="xs")
            cols_sb = cpool.tile([P, 2 * TILES], f32, tag="cols")

            # Small vector loads ride the scalar ring so the sync ring
            # starts streaming sigma immediately.
            nc.scalar.dma_start(out=xs[:], in_=xsvec[0:1, :])
            nc.scalar.dma_start(out=cols_sb[:], in_=cols[:, :])
            # Prefetch all sigma tiles up front (bufs=4 -> no slot waits).
            s_tiles = []
            for t in range(TILES):
                s = work.tile([P, N], f32, tag="s", bufs=TILES)
                nc.sync.dma_start(out=s[:], in_=sigma[t * P:(t + 1) * P, :])
                s_tiles.append(s)

            ones = xs[0:1, N:N + P]
            nones = xs[0:1, N + P:N + 2 * P]
            MU0 = N + 2 * P  # offset of mu inside xs

            # Broadcasts live in PSUM, 4 banks each, rebuilt per column
            # phase: w[1,128]^T @ x[1,512] per bank (bitwise exact for
            # fp32, verified on HW).  mu_ps holds -mu_k.
            mu_ps = ppool.tile([P, HALF], f32, tag="mups")
            d_ps = ppool.tile([P, HALF], f32, tag="dps")

            # Full-width staging for mu: each tile takes its two column
            # halves from the two phases, then goes out as one 2 MiB DMA.
            m_tiles = [
                work.tile([P, N], f32, tag="m", bufs=TILES, name=f"m{t}")
                for t in range(TILES)
            ]

            for h in range(2):
                c0 = h * HALF
                # mu broadcast first: the mu pass has no sigma dep, so
                # its stores feed the scalar ring earliest.
                for c in range(HALF // BANK):
                    nc.tensor.matmul(
                        mu_ps[:, c * BANK:(c + 1) * BANK], nones,
                        xs[0:1, MU0 + c0 + c * BANK:MU0 + c0 + (c + 1) * BANK],
                        start=True, stop=True,
                    )
                for c in range(HALF // BANK):
                    nc.tensor.matmul(
                        d_ps[:, c * BANK:(c + 1) * BANK], ones,
                        xs[0:1, c0 + c * BANK:c0 + (c + 1) * BANK],
                        start=True, stop=True,
                    )

                # mu pass on DVE: m = (-mu_k) + mu_j; (-mu_k) is exact
                # so this rounds identically to mu_j - mu_k.  (GpSimd
                # cannot read PSUM, so Vector carries both passes.)
                for t in range(TILES):
                    nc.vector.tensor_scalar_add(
                        m_tiles[t][:, c0:c0 + HALF], mu_ps[:],
                        cols_sb[:, TILES + t:TILES + t + 1],
                    )
                    if h == 1:
                        nc.scalar.dma_start(
                            out=mu_out[t * P:(t + 1) * P, :], in_=m_tiles[t][:]
                        )

                # sig pass: T = d_k + d_j (from PSUM), then one fused DVE
                # op sig = (S * -2) + T; -2*S is exact and T + (-2S)
                # rounds identically to T - 2S, so this stays bitwise
                # equal to the reference.
                for t in range(TILES):
                    tt = work.tile([P, HALF], f32, tag="tt", bufs=4)
                    nc.scalar.activation(
                        tt[:], d_ps[:], ident,
                        bias=cols_sb[:, t:t + 1], scale=1.0,
                    )
                    nc.vector.scalar_tensor_tensor(
                        tt[:], s_tiles[t][:, c0:c0 + HALF], -2.0, tt[:],
                        op0=mybir.AluOpType.mult, op1=mybir.AluOpType.add,
                    )
                    # sig stores ride the sync ring, which is done
                    # loading by the time these are ready.
                    nc.sync.dma_start(
                        out=sig_out[t * P:(t + 1) * P, c0:c0 + HALF], in_=tt[:]
                    )

    return nc


def _get_program():
    global _PROGRAM
    if _PROGRAM is None:
        nc = _build_program()
        # Bacc defers register allocation / wait splitting to finalize();
        # the axon PJRT path serializes the module as-is, so run it here.
        nc.finalize()
        _PROGRAM = nc
    return _PROGRAM


def _make_in_maps(mu, Sigma, d):
    xsvec = np.concatenate(
        [d, np.ones(P, np.float32), -np.ones(P, np.float32), mu]
    ).reshape(1, 2 * N + 2 * P)
    in_maps = []
    for c in range(NCORES):
        j0 = c * RPC
        cols = np.concatenate(
            [
                d[j0:j0 + RPC].reshape(TILES, P).T,
                mu[j0:j0 + RPC].reshape(TILES, P).T,
            ],
            axis=1,
        )
        in_maps.append({
            "sigma_rows": np.ascontiguousarray(Sigma[j0:j0 + RPC]),
            "xsvec": xsvec,
            "cols": np.ascontiguousarray(cols),
        })
    return in_maps


def _assemble(per_core_results, mu, d):
    mu_full = np.concatenate(
        [per_core_results[c]["mu_out"] for c in range(NCORES)], axis=0
    )
    sig_full = np.concatenate(
        [per_core_results[c]["sig_out"] for c in range(NCORES)], axis=0
    )
    idx = np.arange(N)
    mu_full[idx, idx] = -mu
    sig_full[idx, idx] = d
    return mu_full.reshape(-1), sig_full.reshape(-1)


def kernel(mu, Sigma, _trace=False):
    from concourse.bass_utils import run_bass_kernel_spmd

    mu = np.ascontiguousarray(np.asarray(mu, dtype=np.float32).reshape(N))
    Sigma = np.ascontiguousarray(np.asarray(Sigma, dtype=np.float32).reshape(N, N))
    d = np.ascontiguousarray(np.diagonal(Sigma)).astype(np.float32)

    nc = _get_program()
    in_maps = _make_in_maps(mu, Sigma, d)
    res = run_bass_kernel_spmd(nc, in_maps, list(range(NCORES)), trace=_trace)
    out = _assemble(res.results, mu, d)
    if _trace:
        return out, res
    return out
